# revision 43
# baseline (speedup 1.0000x reference)
"""CostVolumeLayer Trainium2 kernel.

Computes the local cost volume: for search_range R=4,
  out[b, di*9+dj, i, j] = sum_c src[b,c,i,j] * tgt_zp[b,c,i-2R+di, j-2R+dj]
(tgt zero-padded outside its bounds; off-center window, faithful to the
torch reference).

Strategy (8 NeuronCores, SPMD):
  - Shard: core c -> batch b = c//2, row-half r0 = 32*(c%2).
  - Device: per 8x16 pixel block, TWO M=64 TensorE matmuls build the
    banded Gram in ONE psum bank using PE tiling (tile_position inferred
    from the output partition offset):
      A: stationary = pixels mi<4  [C,64] -> psum partitions 0..63,
         rhs = tgt window rows [8bi, 8bi+12) x 24 cols (N=288)
      B: stationary = pixels mi>=4 [C,64] -> psum partitions 64..127,
         rhs = rows [8bi+4, 8bi+16) x 24
    Lower pixels get Gram cols for window rows 0..11, upper rows 4..15
    (band shifted by -96): the banded layout at full partition width
    with NO extra input bytes and NO psum accumulation pairing.
  - The DMA fabric (~430 GB/s TOTAL, shared by every queue; a single
    HWDGE queue sustains only ~250 GB/s) is the hard constraint:
    2.44 MB in + 2.36 MB out per core. Input chunks are interleaved
    across BOTH HWDGE queues (sync + scalar) in consumption order so
    each queue's FIFO completes in need order and the pair saturates
    the fabric; outputs are spread across sync, scalar, and the
    GpSimd SWDGE.
  - PSUM: 16 2-bank tiles (2 blocks each), pool bufs=4. One fp32->fp16
    copy per tile (~0.74us); stage s = tiles 2s,2s+1 copied by the SAME
    engine (s%2: DVE/ACT) into one SBUF stage [128,1152] -> one output
    DMA (writers of one stage serialize anyway; same-engine avoids the
    cross-engine ordering stall). Last two tiles use separate stages +
    engines + HWDGE queues for a parallel tail.
  - PE warm-up (8 dummy matmuls) bridges until the first chunks land so
    real matmuls start at full clock and never stall.
  - Host: zero-FLOP banded-diagonal gather (band layout as baseline).
"""

import numpy as np

R = 4
D = 2 * R + 1          # 9
B, C, H, W = 4, 128, 64, 128
NCORES = 8
HS = H // 2            # 32 rows per core shard
TH = HS + 2 * R        # 40 padded tgt rows per shard
TW = W + 2 * R         # 136 padded tgt cols
BI, BJ = 8, 16         # pixel block: 8 rows x 16 cols = 128 pixels
NBI, NBJ = HS // BI, W // BJ   # 4 x 8 = 32 blocks per core
WIN_J = BJ + 2 * R     # 24 window cols
NA = 12 * WIN_J        # 288 streamed cols per half-matmul
BANDW = NA             # 288 band cols dumped per pixel
BANDO = 4 * WIN_J      # 96, upper-half band column offset (host gather)
SRCC = NBI * NBJ * 128  # 4096 src cols
TGT0 = SRCC
TGTC = TH * TW          # 5440
E = TGT0 + TGTC         # 9536 input cols per partition
PSB = 512              # fp32 elems per PSUM bank (2KB)
TPB = 2                # blocks (banks) per psum tile
NPT = NBI * NBJ // TPB  # 16 psum tiles
STGW = 4 * BANDW       # 1152 fp16 cols per stage (2 psum tiles)
NST = 8                # output stages

_compiled = None


def _build_bass():
    import concourse.mybir as mybir
    from concourse import bacc
    from concourse.tile import TileContext

    f32 = mybir.dt.float32
    in_dt = mybir.dt.bfloat16
    dump_dt = mybir.dt.float16
    nc = bacc.Bacc()
    inp = nc.dram_tensor("inp", [C, E], in_dt, kind="ExternalInput")
    gout = nc.dram_tensor("gout", [NST, 128, STGW], dump_dt,
                          kind="ExternalOutput")
    gout_ap = gout.ap()

    with TileContext(nc) as tc:
        with (
            tc.tile_pool(name="inp", bufs=1) as inp_pool,
            tc.tile_pool(name="g", bufs=NST) as gpool,
            tc.tile_pool(name="psum", bufs=4, space="PSUM") as psum_pool,
        ):
            a = inp_pool.tile([C, E], in_dt)

            def t_view():
                return a[:, TGT0:].rearrange("c (i j) -> c i j", j=TW)

            def new_pt():
                return psum_pool.tile([128, TPB * PSB], f32, name="pt")

            warm = inp_pool.tile([128, PSB], in_dt)
            nc.gpsimd.memset(warm, 0.0)
            # PE warm-up: bridges until the first input chunks land
            # (~11us) while ramping the HAM clock gate to full speed.
            wps = new_pt()
            for _ in range(8):
                nc.tensor.matmul(wps[0:1, 0:PSB], warm[:, :1], warm,
                                 start=True, stop=True)
            # ACT warm-up: first Activation op loads the activation table.
            actwarm = inp_pool.tile([1, 1], dump_dt)
            nc.scalar.copy(actwarm, warm[0:1, 0:1])

            # Input DMAs: one HWDGE queue sustains only ~250 GB/s; the
            # ~430 GB/s fabric needs BOTH queues pulling. Interleave the
            # consumption-ordered chunks across sync and scalar so each
            # queue's FIFO still completes in need order.
            iv = inp.ap()

            def chunk(q, lo, hi):
                q.dma_start(out=a[:, lo:hi], in_=iv[:, lo:hi])

            def tchunk(q, r0, r1):
                chunk(q, TGT0 + r0 * TW, TGT0 + r1 * TW)

            chunk(nc.sync, 0, 8 * 128)        # blocks 0-7   (tiles 0-3)
            tchunk(nc.scalar, 0, 12)          # tgt rows 0-11 (bi=0 A)
            tchunk(nc.sync, 12, 16)           # rows 12-15    (bi=0 B)
            tchunk(nc.scalar, 16, 28)         # rows 16-27    (bi=1, bi=2 A)
            chunk(nc.sync, 8 * 128, 16 * 128)     # blocks 8-15 (t4-7)
            chunk(nc.scalar, 16 * 128, 24 * 128)  # blocks 16-23 (t8-11)
            tchunk(nc.sync, 28, 40)           # rows 28-39    (bi=2 B, bi=3)
            chunk(nc.sync, 24 * 128, TGT0)        # blocks 24-31 (t12-15)

            half = STGW // 2
            for t in range(NPT):
                bi = t // 4
                pt = wps if t == 0 else new_pt()
                ptv = pt.rearrange("p (b h) -> p b h", b=TPB)
                for j in range(TPB):
                    blk = t * TPB + j
                    bj = blk % NBJ
                    sb = blk * 128
                    rhsA = t_view()[:, bi * BI: bi * BI + 12,
                                    bj * BJ: bj * BJ + WIN_J]
                    rhsB = t_view()[:, bi * BI + 4: bi * BI + 16,
                                    bj * BJ: bj * BJ + WIN_J]
                    nc.tensor.matmul(ptv[0:64, j, :NA], a[:, sb:sb + 64],
                                     rhsA, start=True, stop=True)
                    nc.tensor.matmul(ptv[64:128, j, :NA],
                                     a[:, sb + 64:sb + 128],
                                     rhsB, start=True, stop=True)
                s, k = divmod(t, 2)
                if t < NPT - 4:
                    if k == 0:
                        stage = gpool.tile([128, STGW], dump_dt)
                        stages = stage
                    eng = (nc.vector.tensor_copy if s % 2 == 0
                           else nc.scalar.copy)
                    eng(stages[:, k * half:(k + 1) * half]
                        .rearrange("p (b w) -> p b w", b=TPB),
                        ptv[:, :, 0:BANDW])
                    if k == 1:
                        # outputs spread across all three DMA paths
                        # (SWDGE gen is ~1.3us/stage; HWDGE queues free
                        # up once the input FIFOs drain)
                        q = (nc.gpsimd, nc.scalar, nc.sync,
                             nc.gpsimd, nc.sync, nc.sync)[s]
                        q.dma_start(out=gout_ap[s], in_=stages)
                else:
                    # tail-latency: the last FOUR psum tiles get per-tile
                    # stages on alternating engines and their own DMAs
                    # fanned across all three queues, so the end-of-kernel
                    # output drain runs fully in parallel
                    stgt = gpool.tile([128, half], dump_dt, name="stgt")
                    eng = (nc.vector.tensor_copy if t % 2 == 0
                           else nc.scalar.copy)
                    eng(stgt.rearrange("p (b w) -> p b w", b=TPB),
                        ptv[:, :, 0:BANDW])
                    q = {NPT - 4: nc.gpsimd, NPT - 3: nc.sync,
                         NPT - 2: nc.scalar, NPT - 1: nc.gpsimd}[t]
                    q.dma_start(out=gout_ap[s][:, k * half:(k + 1) * half],
                                in_=stgt)
    nc.finalize()
    return nc


def _get_compiled():
    global _compiled
    if _compiled is None:
        _compiled = _build_bass()
    return _compiled


def _shard_inputs(src, tgt):
    """Per-core input maps: block-reordered src + zero-padded tgt halo."""
    import ml_dtypes

    bf16 = ml_dtypes.bfloat16
    in_maps = []
    for c in range(NCORES):
        b = c // 2
        r0 = HS * (c % 2)
        s = (src[b, :, r0:r0 + HS, :]
             .reshape(C, NBI, BI, NBJ, BJ)
             .transpose(0, 1, 3, 2, 4)
             .reshape(C, SRCC))
        tp = np.zeros((C, TH, TW), dtype=np.float32)
        lo = r0 - 2 * R
        hi = r0 + HS
        clo = max(lo, 0)
        tp[:, clo - lo: clo - lo + (hi - clo), 2 * R: 2 * R + W] = \
            tgt[b, :, clo:hi, :]
        inp = np.concatenate([s, tp.reshape(C, TGTC)], axis=1)
        in_maps.append({"inp": np.ascontiguousarray(inp.astype(bf16))})
    return in_maps


# host-side gather indices: out[k=(di,dj)] at pixel (mi,mj) of a block sits
# at band col n = (mi+di)*WIN_J + (mj+dj), shifted by BANDO for mi >= 4.
_mi = np.arange(BI)[:, None, None, None]
_mj = np.arange(BJ)[None, :, None, None]
_di = np.arange(D)[None, None, :, None]
_dj = np.arange(D)[None, None, None, :]
_NIDX = ((_mi + _di) * WIN_J + (_mj + _dj)
         - BANDO * (_mi >= 4)).reshape(BI, BJ, D * D)  # [8,16,81]


def _unshard_output(results):
    out = np.empty((B, D * D, H, W), dtype=np.float32)
    for c in range(NCORES):
        b = c // 2
        r0 = HS * (c % 2)
        g = (results[c]["gout"]
             .astype(np.float32)
             .reshape(NBI, 2, 128, 4, BANDW)   # [bi, stage-half, p, j, w]
             .transpose(0, 1, 3, 2, 4)
             .reshape(NBI, NBJ, BI, BJ, BANDW))
        v = np.take_along_axis(g, _NIDX[None, None], axis=-1)
        v = v.transpose(4, 0, 2, 1, 3)  # [81, NBI, BI, NBJ, BJ]
        out[b, :, r0:r0 + HS, :] = v.reshape(D * D, HS, W)
    return out


def kernel(src, tgt):
    from concourse.bass_utils import run_bass_kernel_spmd

    src = np.asarray(src, dtype=np.float32)
    tgt = np.asarray(tgt, dtype=np.float32)
    nc = _get_compiled()
    in_maps = _shard_inputs(src, tgt)
    res = run_bass_kernel_spmd(nc, in_maps, core_ids=list(range(NCORES)))
    return _unshard_output(res.results)
